# revision 14
# baseline (speedup 1.0000x reference)
"""Trainium2 Bass kernel for nn_DivEncLayer (grouped per-slice MLP 8->32->1).

Reference computation (per batch row b, per slice q of 128):
    xs = x.reshape(B, 128, 8)
    h  = ELU(xs[b,q,:] @ W1[q] + b1[q])            # (32,)
    h  = (h - mov_mean[q]) * gamma[q]/sqrt(mov_var[q]+eps) + beta[q]
    out[b,q] = h @ W2[q] + b2[q]

Strategy (pure data parallel over 8 NeuronCores, B=32768 -> 4096/core):
  * BN affine + W2 fold into w2p[q,h] (host); final bias bfin[q] (host).
  * ELU(u) = ReLU(u) + min(exp(u),1) - 1  (exact identity), so
       out[q,b] = sum_h w2p*ReLU(u) + sum_h w2p*min(e^u,1) + bfin[q].
  * On-chip per core:
      - PE transposes x tiles ([128b,128c] -> [128c,128b]) via identity matmul
      - dense1: 16 tile_position-packed matmuls per 16-slice group
        (each 32x32 array tile computes one slice's 8->32 matmul, K_eff=8)
      - ACT drains PSUM: Exp pass (+ Relu pass on even groups);
        DVE does Relu drain on odd groups + min(e,1) + copies
      - dense2: 32 packed matmuls (ReLU part + min part) accumulate into one
        PSUM bank laid out so partition index == slice index q
      - bias add, PE transpose back to [b,q], DMA out
"""

import sys

for _p in ("/opt/trn_rl_repo", "/root/.axon_site/_ro/trn_rl_repo"):
    if _p not in sys.path:
        sys.path.append(_p)

import numpy as np

import concourse.bass as bass
import concourse.tile as tile
from concourse import mybir
from concourse.bass_utils import run_bass_kernel_spmd
from concourse.masks import make_identity

F32 = mybir.dt.float32

_NOPN = [0]


def _split_matmul_waits(tc):
    """walrus only supports one sync-wait command on several instruction
    encodings (PE Matmult with inline fp32 LDW, DMA direct2d, ...).  Tile can
    emit several.  Wrap the TileContext instruction-append hook so every
    multi-wait instruction is preceded by a same-engine NoOp carrying all but
    the last wait."""
    orig = tc._add_instruction

    def patched(inst):
        si = inst.sync_info
        if (
            not isinstance(inst, mybir.InstNoOp)
            and si is not None
            and len(si.on_wait) > 1
        ):
            for w in si.on_wait[:-1]:
                _NOPN[0] += 1
                nop = mybir.InstNoOp(name=f"waitnop-{_NOPN[0]}", ins=[], outs=[])
                nop.engine = inst.engine
                nop.sync_info = mybir.SyncInfo(on_wait=[w], on_update=[])
                orig(nop)
            inst.sync_info = mybir.SyncInfo(
                on_wait=[si.on_wait[-1]], on_update=list(si.on_update)
            )
        return orig(inst)

    tc._add_instruction = patched

    def patched_dab(tick_clock, wait_clock):
        from concourse.vector_clock import ScopedClock

        nc = tc.nc
        drain_inst = nc.sync.drain()
        wait_clock.add_sem_waits(
            drain_inst.ins, ScopedClock({None: tick_clock.global_clock})
        )
        si = drain_inst.ins.sync_info
        if si is not None and len(si.on_wait) > 1:
            extra = list(si.on_wait[1:])
            drain_inst.ins.sync_info = mybir.SyncInfo(
                on_wait=[si.on_wait[0]], on_update=list(si.on_update)
            )
            for w in extra:
                n = nc.sync.nop(nofuse=True)
                n.ins.sync_info = mybir.SyncInfo(on_wait=[w], on_update=[])

        nc.all_engine_barrier()
        assert tc.sems is not None
        popped = nc._tile_sem_poison_stack.pop()
        assert popped is tc._sem_poison
        nc.clear_and_free_semaphores(list(tc.sems.allocated().values()))
        nc.all_engine_barrier()

    tc._drain_and_barrier = patched_dab

Q, S, H = 128, 8, 32
C = Q * S                      # 1024
B_FULL = 32768
NCORES = 8
BN_EPS = 1e-3

NB = 512                       # batch tile (matmul free dim)
NG = 8                         # c/slice groups of 16 slices (128 partitions)
RELU_ON_ACT = True             # ReLU drain engine: ACT (True) or DVE (False)


def _host_pack(W1, b1, gamma, beta, mov_mean, mov_var, W2, b2):
    """Fold BN into second dense; pack block weights for the PE layouts."""
    W1 = np.asarray(W1, np.float32).reshape(Q, S, H)
    b1 = np.asarray(b1, np.float32).reshape(Q, H)
    gamma = np.asarray(gamma, np.float32).reshape(Q, H)
    beta = np.asarray(beta, np.float32).reshape(Q, H)
    mean = np.asarray(mov_mean, np.float32).reshape(Q, H)
    var = np.asarray(mov_var, np.float32).reshape(Q, H)
    W2 = np.asarray(W2, np.float32).reshape(Q, H)
    b2 = np.asarray(b2, np.float32).reshape(Q)

    inv = gamma / np.sqrt(var + BN_EPS)
    w2p = (inv * W2).astype(np.float32)                      # [Q,H]
    # out = sum_h w2p*(ReLU(u) + min(e^u,1)) + bfin
    # bfin = b2 + sum_h (beta - mean*inv)*W2 - sum_h w2p   (the -1 of min-1)
    bfin = (b2 + ((beta - mean * inv) * W2).sum(-1) - w2p.sum(-1)).astype(np.float32)

    # dense1 stationary blocks: tile (g,i,j) computes slice q=16g+4i+j.
    # lhsT rows live at partitions 32i+8j..32i+8j+8 (c offsets of slice q),
    # cols 32j..32j+32 (h outputs, psum col group j).
    w1bd = np.zeros((128, NG, 128), np.float32)
    for g in range(NG):
        for i in range(4):
            for j in range(4):
                q = 16 * g + 4 * i + j
                w1bd[32 * i + 8 * j:32 * i + 8 * j + 8, g, 32 * j:32 * j + 32] = W1[q]

    # dense2 stationary blocks: tile (g,i,j) contracts h (rows 32j..32j+32 of
    # the elementwise tiles, bank i) against w2p[q], output partition
    # 32*(g//2) + m with m = 16*(g%2)+4i+j  (== partition 16g+4i+j == q).
    w2t = np.zeros((128, NG, 4, 32), np.float32)
    for g in range(NG):
        for i in range(4):
            for j in range(4):
                q = 16 * g + 4 * i + j
                m = 16 * (g % 2) + 4 * i + j
                w2t[32 * j:32 * j + 32, g, i, m] = w2p[q]

    # per-partition b1 for the (rare) b1 != 0 path: [p=32j+h, g, i]
    b1sb = np.zeros((128, NG, 4, 1), np.float32)
    for g in range(NG):
        for i in range(4):
            for j in range(4):
                q = 16 * g + 4 * i + j
                b1sb[32 * j:32 * j + 32, g, i, 0] = b1[q]

    return w1bd, w2t, bfin.reshape(128, 1), b1sb, bool(np.any(b1 != 0.0))


def _build(bc, has_b1):
    """Build the Bass program for one core processing bc batch rows."""
    nc = bass.Bass()

    x_d = nc.dram_tensor("x", [bc, C], F32, kind="ExternalInput")
    w1_d = nc.dram_tensor("w1bd", [128, NG, 128], F32, kind="ExternalInput")
    w2_d = nc.dram_tensor("w2t", [128, NG, 4, 32], F32, kind="ExternalInput")
    bf_d = nc.dram_tensor("bfin", [128, 1], F32, kind="ExternalInput")
    b1_d = nc.dram_tensor("b1sb", [128, NG, 4, 1], F32, kind="ExternalInput")
    out_d = nc.dram_tensor("out", [bc, 128], F32, kind="ExternalOutput")

    n_tiles = bc // NB
    Relu = mybir.ActivationFunctionType.Relu
    Exp = mybir.ActivationFunctionType.Exp

    with tile.TileContext(nc) as tc:
        _split_matmul_waits(tc)
        with (
            tc.tile_pool(name="singles", bufs=1) as singles,
            tc.tile_pool(name="xnat", bufs=6) as xnat_pool,
            tc.tile_pool(name="xt", bufs=10) as xt_pool,
            tc.tile_pool(name="mid", bufs=2) as mid_pool,
            tc.tile_pool(name="outq", bufs=2) as outq_pool,
            tc.tile_pool(name="outb", bufs=2) as outb_pool,
            tc.tile_pool(name="ps_u", bufs=1, space="PSUM") as ps_u,
            tc.tile_pool(name="ps_o", bufs=2, space="PSUM") as ps_o,
            tc.tile_pool(name="ps_t", bufs=2, space="PSUM") as ps_t,
        ):
            w1t = singles.tile([128, NG, 128], F32)
            w2t = singles.tile([128, NG, 4, 32], F32)
            bfin = singles.tile([128, 1], F32)
            b1sb = singles.tile([128, NG, 4, 1], F32)
            ident = singles.tile([128, 128], F32)

            zbias = singles.tile([128, 1], F32)
            wdum = singles.tile([128, 128], F32)

            nc.sync.dma_start(w1t[:], w1_d[:])
            nc.sync.dma_start(w2t[:], w2_d[:])
            nc.sync.dma_start(bfin[:], bf_d[:])
            nc.sync.dma_start(b1sb[:], b1_d[:])
            make_identity(nc, ident[:])
            nc.gpsimd.memset(zbias[:], 0.0)

            # Warmup: PE matmuls (fp32 self-loading LDW) only support a single
            # sync wait, and Tile emits a wait per not-yet-observed producer.
            # Touch each one-time producer (identity, weight DMAs, zero bias)
            # from each consuming engine once, so steady-state instructions
            # need at most one wait each.
            pdum = ps_t.tile([128, 4, 128], F32, tag="tp")
            nc.tensor.transpose(pdum[0:1, 0, :], ident[:, 0:1], ident[:])
            nc.tensor.transpose(pdum[0:1, 1, :], w1t[:, 0, 0:1], ident[:])
            nc.tensor.transpose(pdum[0:1, 2, :], w2t[:, 0, 0, 0:1], ident[:])
            nc.vector.tensor_copy(wdum[:, 0:1], bfin[:])
            nc.scalar.activation(wdum[:, 1:2], zbias[:], mybir.ActivationFunctionType.Relu)
            if has_b1:
                nc.scalar.activation(wdum[:, 2:3], b1sb[:, 0, 0, :], mybir.ActivationFunctionType.Relu)

            for n in range(n_tiles):
                # ---- load 512 batch rows as 4 tiles of [128, 1024]
                xns = []
                for k in range(4):
                    xn = xnat_pool.tile([128, C], F32, tag="xnat")
                    nc.sync.dma_start(xn[:], x_d[NB * n + 128 * k:NB * n + 128 * (k + 1), :])
                    xns.append(xn)

                outq = outq_pool.tile([128, NB], F32, tag="outq")

                rts = {}
                for g in range(NG):
                    # ---- transpose this c-group: [128b,128c] -> [128c,128b]
                    tp = ps_t.tile([128, 4, 128], F32, tag="tp")
                    for k in range(4):
                        nc.tensor.transpose(tp[:, k, :], xns[k][:, 128 * g:128 * (g + 1)], ident[:])
                    xt = xt_pool.tile([128, 4, 128], F32, tag="xt")
                    nc.vector.tensor_copy(xt[:], tp[:])

                    # ---- dense1: 16 packed matmuls -> u[p=32j+h, bank i, b]
                    u = ps_u.tile([128, 4, NB], F32, tag="u")
                    for i in range(4):
                        for j in range(4):
                            nc.tensor.matmul(
                                u[32 * j:32 * j + 32, i, :],
                                w1t[32 * i:32 * i + 32, g, 32 * j:32 * j + 32],
                                xt[32 * i:32 * i + 32, :, :],
                                start=True,
                                stop=True,
                                tile_position=(32 * i, 32 * j),
                            )

                    # ---- elementwise: R = relu(u+b1), E = exp(u+b1), T = min(E,1)
                    ew = mid_pool.tile([128, 4, NB], F32, tag="E")
                    rw = mid_pool.tile([128, 4, NB], F32, tag="R")
                    tw = mid_pool.tile([128, 4, NB], F32, tag="T")
                    if has_b1:
                        for i in range(4):
                            nc.scalar.activation(ew[:, i, :], u[:, i, :], Exp, bias=b1sb[:, g, i, :])
                            if RELU_ON_ACT:
                                nc.scalar.activation(rw[:, i, :], u[:, i, :], Relu, bias=b1sb[:, g, i, :])
                            else:
                                nc.vector.tensor_scalar(
                                    rw[:, i, :], u[:, i, :],
                                    scalar1=b1sb[:, g, i, :], scalar2=0.0,
                                    op0=mybir.AluOpType.add, op1=mybir.AluOpType.max,
                                )
                    else:
                        nc.scalar.activation(ew[:], u[:], Exp, bias=zbias[:])
                        if RELU_ON_ACT:
                            nc.scalar.activation(rw[:], u[:], Relu, bias=zbias[:])
                        else:
                            nc.vector.tensor_scalar_max(rw[:], u[:], 0.0)
                    nc.vector.tensor_scalar_min(tw[:], ew[:], 1.0)
                    rts[g] = (rw, tw)

                    # ---- dense2 for the (even, odd) pair: 16 standard K=128
                    # matmuls (block-diagonal lhsT, 4 live cols each)
                    # accumulate into o[32c:32c+32]; partition 16g'+4i+j == q.
                    if g % 2 == 1:
                        o = ps_o.tile([128, NB], F32, tag="o")
                        base = 32 * (g // 2)
                        mms = [
                            (gp, i, t)
                            for gp in (g - 1, g)
                            for i in range(4)
                            for t in (0, 1)
                        ]
                        for kseq, (gp, i, t) in enumerate(mms):
                            nc.tensor.matmul(
                                o[base:base + 32, :],
                                w2t[:, gp, i, :],
                                rts[gp][t][:, i, :],
                                start=(kseq == 0),
                                stop=(kseq == len(mms) - 1),
                                tile_position=(0, base),
                            )
                        rts.clear()

                        # ---- bias add; partition 16g+p == slice index q
                        nc.vector.tensor_scalar_add(
                            outq[base:base + 32, :],
                            o[base:base + 32, :],
                            bfin[base:base + 32, :],
                        )

                # ---- transpose [128q, 512b] -> 4x [128b, 128q], store
                ot = ps_t.tile([128, 4, 128], F32, tag="tp")
                for k in range(4):
                    nc.tensor.transpose(ot[:, k, :], outq[:, 128 * k:128 * (k + 1)], ident[:])
                ob = outb_pool.tile([128, 4, 128], F32, tag="outb")
                nc.vector.tensor_copy(ob[:], ot[:])
                nc.sync.dma_start(
                    out_d[NB * n:NB * (n + 1), :].rearrange("(k p) q -> p k q", p=128),
                    ob[:],
                )

    return nc


_CACHE = {}


def _get_nc(bc, has_b1):
    key = (bc, has_b1)
    if key not in _CACHE:
        _CACHE[key] = _build(bc, has_b1)
    return _CACHE[key]


def kernel(x, W1, b1, gamma, beta, mov_mean, mov_var, W2, b2):
    x = np.asarray(x, np.float32).reshape(-1, C)
    B = x.shape[0]
    w1bd, w2t, bfin, b1sb, has_b1 = _host_pack(
        W1, b1, gamma, beta, mov_mean, mov_var, W2, b2
    )

    bc = B // NCORES
    nc = _get_nc(bc, has_b1)

    in_maps = [
        {
            "x": np.ascontiguousarray(x[i * bc:(i + 1) * bc]),
            "w1bd": w1bd,
            "w2t": w2t,
            "bfin": bfin,
            "b1sb": b1sb,
        }
        for i in range(NCORES)
    ]
    res = run_bass_kernel_spmd(nc, in_maps, list(range(NCORES)))
    kernel._last_results = res
    out = np.concatenate([res.results[i]["out"] for i in range(NCORES)], axis=0)
    return out.astype(np.float32)


# revision 21
# speedup vs baseline: 1278.8799x; 1278.8799x over previous
"""Trainium2 Bass kernel for nn_DivEncLayer (grouped per-slice MLP 8->32->1).

Reference computation (per batch row b, per slice q of 128):
    xs = x.reshape(B, 128, 8)
    h  = ELU(xs[b,q,:] @ W1[q] + b1[q])            # (32,)
    h  = (h - mov_mean[q]) * gamma[q]/sqrt(mov_var[q]+eps) + beta[q]
    out[b,q] = h @ W2[q] + b2[q]

Strategy (pure data parallel over 8 NeuronCores, B=32768 -> 4096/core):
  * BN affine + W2 fold into w2p[q,h] (host); final bias bfin[q] (host).
  * ELU(u) = ReLU(u) + min(exp(u),1) - 1  (exact identity), so
       out[q,b] = sum_h w2p*ReLU(u) + sum_h w2p*min(e^u,1) + bfin[q].
  * On-chip per core, per batch tile of 512 and slice group of 16:
      - PE transposes x tiles ([128b,128c] -> [128c,128b]) via identity matmul
      - dense1: 16 tile_position-packed matmuls (each 32x32 array tile
        computes one slice's 8->32 matmul) into 2 half-u PSUM tiles
      - ACT Exp pass drains PSUM; ReLU drain alternates ACT/DVE; min(E,1)
        runs on GpSimd; all mid tensors bf16 (PE fast dtype)
      - dense2: 16 standard K=128 matmuls per group pair (block-diagonal
        lhsT) accumulating into one PSUM bank; partition index == q
      - bias add (DVE), PE transpose back to [b,q], DMA out
  * PE operands are float32r (x path) / bf16 (mid path): 1 cycle/row vs 4
    for plain fp32.

Known walrus/HW constraints handled here:
  * any instruction encoding supports only ONE semaphore wait -> _split_waits
  * PSUM accumulation chains must share one tile_position
  * matmul PSUM output base partition must be 32-aligned
"""

import sys

for _p in ("/opt/trn_rl_repo", "/root/.axon_site/_ro/trn_rl_repo"):
    if _p not in sys.path:
        sys.path.append(_p)

import contextlib

import numpy as np

import concourse.bass as bass
import concourse.tile as tile
from concourse import mybir
from concourse.bass_utils import run_bass_kernel_spmd
from concourse.masks import make_identity

F32 = mybir.dt.float32
F32R = mybir.dt.float32r
BF16 = mybir.dt.bfloat16

Q, S, H = 128, 8, 32
C = Q * S                      # 1024
NCORES = 8
BN_EPS = 1e-3

NB = 512                       # batch tile (matmul free dim)
NG = 8                         # c/slice groups of 16 slices (128 partitions)

MID_DT = BF16                  # dtype of E/R/T elementwise tensors
RELU_ACT_MOD = 2               # relu on ACT when g % RELU_ACT_MOD == 0
MIN_ON_GPSIMD = True

_NOPN = [0]


def _split_waits(tc):
    """walrus supports only one sync-wait command per instruction; Tile can
    emit several.  Precede every multi-wait instruction with same-engine
    NoOps carrying all but the last wait."""
    orig = tc._add_instruction

    def patched(inst):
        si = inst.sync_info
        if (
            not inst.name.startswith("waitnop")
            and si is not None
            and len(si.on_wait) > 1
        ):
            for w in si.on_wait[:-1]:
                _NOPN[0] += 1
                nop = mybir.InstNoOp(name=f"waitnop-{_NOPN[0]}", ins=[], outs=[])
                nop.engine = inst.engine
                nop.sync_info = mybir.SyncInfo(on_wait=[w], on_update=[])
                orig(nop)
            inst.sync_info = mybir.SyncInfo(
                on_wait=[si.on_wait[-1]], on_update=list(si.on_update)
            )
        return orig(inst)

    tc._add_instruction = patched

    def patched_dab(tick_clock, wait_clock):
        from concourse.vector_clock import ScopedClock

        nc = tc.nc
        drain_inst = nc.sync.drain()
        wait_clock.add_sem_waits(
            drain_inst.ins, ScopedClock({None: tick_clock.global_clock})
        )
        si = drain_inst.ins.sync_info
        if si is not None and len(si.on_wait) > 1:
            extra = list(si.on_wait[1:])
            drain_inst.ins.sync_info = mybir.SyncInfo(
                on_wait=[si.on_wait[0]], on_update=list(si.on_update)
            )
            for w in extra:
                n = nc.sync.nop(nofuse=True)
                n.ins.sync_info = mybir.SyncInfo(on_wait=[w], on_update=[])

        nc.all_engine_barrier()
        assert tc.sems is not None
        popped = nc._tile_sem_poison_stack.pop()
        assert popped is tc._sem_poison
        nc.clear_and_free_semaphores(list(tc.sems.allocated().values()))
        nc.all_engine_barrier()

    tc._drain_and_barrier = patched_dab


def _host_pack(W1, b1, gamma, beta, mov_mean, mov_var, W2, b2):
    """Fold BN into second dense; pack block weights for the PE layouts."""
    import ml_dtypes

    W1 = np.asarray(W1, np.float32).reshape(Q, S, H)
    b1 = np.asarray(b1, np.float32).reshape(Q, H)
    gamma = np.asarray(gamma, np.float32).reshape(Q, H)
    beta = np.asarray(beta, np.float32).reshape(Q, H)
    mean = np.asarray(mov_mean, np.float32).reshape(Q, H)
    var = np.asarray(mov_var, np.float32).reshape(Q, H)
    W2 = np.asarray(W2, np.float32).reshape(Q, H)
    b2 = np.asarray(b2, np.float32).reshape(Q)

    inv = gamma / np.sqrt(var + BN_EPS)
    w2p = (inv * W2).astype(np.float32)                      # [Q,H]
    # out = sum_h w2p*(ReLU(u) + min(e^u,1)) + bfin
    bfin = (b2 + ((beta - mean * inv) * W2).sum(-1) - w2p.sum(-1)).astype(np.float32)

    # dense1 stationary blocks: MM (g,i) is a standard K=128 matmul with a
    # block-diagonal lhsT (rows 32i..32i+32 live) computing slices
    # q=16g+4i+j at output partitions 32j+h.  (f32r matmuls require dst
    # partition base 0, so no tile_position col packing.)
    w1bd = np.zeros((128, NG, 4, 128), np.float32)
    for g in range(NG):
        for i in range(4):
            for j in range(4):
                q = 16 * g + 4 * i + j
                w1bd[32 * i + 8 * j:32 * i + 8 * j + 8, g, i, 32 * j:32 * j + 32] = W1[q]

    # dense2 block-diagonal lhsT: col m holds w2p of slice q=16g+4i+j at rows
    # 32j..32j+32, with m = 16*(g%2)+4i+j so output partition == q.
    w2t = np.zeros((128, NG, 4, 32), np.float32)
    for g in range(NG):
        for i in range(4):
            for j in range(4):
                q = 16 * g + 4 * i + j
                m = 16 * (g % 2) + 4 * i + j
                w2t[32 * j:32 * j + 32, g, i, m] = w2p[q]
    if MID_DT == BF16:
        w2t = w2t.astype(ml_dtypes.bfloat16)

    # per-partition b1 for the (rare) b1 != 0 path: [p=32j+h, g, i]
    b1sb = np.zeros((128, NG, 4, 1), np.float32)
    for g in range(NG):
        for i in range(4):
            for j in range(4):
                q = 16 * g + 4 * i + j
                b1sb[32 * j:32 * j + 32, g, i, 0] = b1[q]

    return w1bd, w2t, bfin.reshape(128, 1), b1sb, bool(np.any(b1 != 0.0))


IDENT = np.eye(128, dtype=np.float32)


def _build(bc, has_b1, rep=1):
    """Build the Bass program for one core processing bc batch rows.

    rep>1 wraps the batch loop in a For loop reprocessing the same data
    (benchmarking only: amplifies kernel time over ~90ms axon dispatch)."""
    nc = bass.Bass()

    x_d = nc.dram_tensor("x", [bc, C], F32R, kind="ExternalInput")
    w1_d = nc.dram_tensor("w1bd", [128, NG, 4, 128], F32R, kind="ExternalInput")
    w2_d = nc.dram_tensor("w2t", [128, NG, 4, 32], MID_DT, kind="ExternalInput")
    bf_d = nc.dram_tensor("bfin", [128, 1], F32, kind="ExternalInput")
    b1_d = nc.dram_tensor("b1sb", [128, NG, 4, 1], F32, kind="ExternalInput")
    id_d = nc.dram_tensor("ident", [128, 128], F32R, kind="ExternalInput")
    out_d = nc.dram_tensor("out", [bc, 128], F32, kind="ExternalOutput")

    n_tiles = bc // NB
    Relu = mybir.ActivationFunctionType.Relu
    Exp = mybir.ActivationFunctionType.Exp

    with tile.TileContext(nc) as tc:
        _split_waits(tc)
        with (
            tc.tile_pool(name="singles", bufs=1) as singles,
            tc.tile_pool(name="xnat", bufs=6) as xnat_pool,
            tc.tile_pool(name="xt", bufs=4) as xt_pool,
            tc.tile_pool(name="mide", bufs=3) as mide_pool,
            tc.tile_pool(name="midrt", bufs=4) as midrt_pool,
            tc.tile_pool(name="outq", bufs=2) as outq_pool,
            tc.tile_pool(name="outb", bufs=2) as outb_pool,
            tc.tile_pool(name="ps_u", bufs=3, space="PSUM") as ps_u,
            tc.tile_pool(name="ps_o", bufs=1, space="PSUM") as ps_o,
            tc.tile_pool(name="ps_t", bufs=1, space="PSUM") as ps_t,
        ):
            w1t = singles.tile([128, NG, 4, 128], F32R)
            w2t = singles.tile([128, NG, 4, 32], MID_DT)
            bfin = singles.tile([128, 1], F32)
            b1sb = singles.tile([128, NG, 4, 1], F32)
            ident = singles.tile([128, 128], F32R)
            identf = singles.tile([128, 128], F32)
            zbias = singles.tile([128, 1], F32)
            wdum = singles.tile([128, 128], F32)

            nc.sync.dma_start(w1t[:], w1_d[:])
            nc.sync.dma_start(w2t[:], w2_d[:])
            nc.sync.dma_start(bfin[:], bf_d[:])
            nc.sync.dma_start(b1sb[:], b1_d[:])
            nc.sync.dma_start(ident[:], id_d[:])
            make_identity(nc, identf[:])
            nc.gpsimd.memset(zbias[:], 0.0)

            # Warmup: make each engine observe each one-time producer once so
            # steady-state instructions need at most one semaphore wait.
            pdum = ps_t.tile([128, 4, 128], F32R, tag="tp")
            nc.tensor.transpose(pdum[0:1, 0, :], ident[:, 0:1], ident[:])
            nc.tensor.transpose(pdum[0:1, 1, :], w1t[:, 0, 0, 0:1], ident[:])
            nc.tensor.transpose(pdum[0:1, 2, :].bitcast(F32), identf[:, 0:1], identf[:])
            nc.vector.tensor_copy(wdum[:, 0:1], bfin[:])
            nc.scalar.activation(wdum[:, 1:2], zbias[:], Relu)
            if has_b1:
                nc.scalar.activation(wdum[:, 2:3], b1sb[:, 0, 0, :], Relu)

            loop_cm = tc.For_i(0, rep, 1) if rep > 1 else contextlib.nullcontext()
            with loop_cm:
              for n in range(n_tiles):
                # ---- load 512 batch rows as 4 tiles of [128, 1024]
                xns = []
                for k in range(4):
                    xn = xnat_pool.tile([128, C], F32R, tag="xnat")
                    nc.sync.dma_start(xn[:], x_d[NB * n + 128 * k:NB * n + 128 * (k + 1), :])
                    xns.append(xn)

                outq = outq_pool.tile([128, NB], F32, tag="outq")
                rts = {}

                def dense2_pair(p):
                    # 16 standard matmuls, all tile_position (0, 32p):
                    # accumulation chains must share one position.
                    o = ps_o.tile([128, NB], F32, tag="o")
                    base = 32 * p
                    mms = [
                        (gp, i, t)
                        for gp in (2 * p, 2 * p + 1)
                        for i in range(4)
                        for t in (0, 1)
                    ]
                    for kseq, (gp, i, t) in enumerate(mms):
                        nc.tensor.matmul(
                            o[base:base + 32, :],
                            w2t[:, gp, i, :],
                            rts[gp][t][:, i, :],
                            start=(kseq == 0),
                            stop=(kseq == len(mms) - 1),
                            tile_position=(0, base),
                        )
                    del rts[2 * p], rts[2 * p + 1]
                    # bias add; partition 16g+4i+j == q
                    nc.vector.tensor_scalar_add(
                        outq[base:base + 32, :],
                        o[base:base + 32, :],
                        bfin[base:base + 32, :],
                    )

                for g in range(NG):
                    # ---- transpose this c-group: [128b,128c] -> [128c,128b]
                    tp = ps_t.tile([128, 4, 128], F32R, tag="tp")
                    for k in range(4):
                        nc.tensor.transpose(tp[:, k, :], xns[k][:, 128 * g:128 * (g + 1)], ident[:])
                    xt = xt_pool.tile([128, 4, 128], F32R, tag="xt")
                    nc.vector.tensor_copy(xt[:], tp[:])

                    # ---- dense1: 16 packed matmuls -> two half-u tiles
                    # (i banks 0,1 / 2,3), layout u[p=32j+h, bank, b]
                    ua = ps_u.tile([128, 2, NB], F32, tag="u")
                    ub = ps_u.tile([128, 2, NB], F32, tag="u")
                    for i in range(4):
                        uh = ua if i < 2 else ub
                        nc.tensor.matmul(
                            uh[:, i % 2, :],
                            w1t[:, g, i, :],
                            xt[:, :, :],
                            start=True,
                            stop=True,
                        )

                    # ---- elementwise: E = exp(u+b1); R = relu(u+b1);
                    #      T = min(E, 1)
                    ew = mide_pool.tile([128, 4, NB], MID_DT, tag="E")
                    rw = midrt_pool.tile([128, 4, NB], MID_DT, tag="R")
                    tw = midrt_pool.tile([128, 4, NB], MID_DT, tag="T")
                    relu_on_act = (g % RELU_ACT_MOD == 0)
                    for hf, uh in ((0, ua), (1, ub)):
                        sl = slice(2 * hf, 2 * hf + 2)
                        if has_b1:
                            for i in (0, 1):
                                bias = b1sb[:, g, 2 * hf + i, :]
                                nc.scalar.activation(ew[:, 2 * hf + i, :], uh[:, i, :], Exp, bias=bias)
                                if relu_on_act:
                                    nc.scalar.activation(rw[:, 2 * hf + i, :], uh[:, i, :], Relu, bias=bias)
                                else:
                                    nc.vector.tensor_scalar(
                                        rw[:, 2 * hf + i, :], uh[:, i, :],
                                        scalar1=bias, scalar2=0.0,
                                        op0=mybir.AluOpType.add, op1=mybir.AluOpType.max,
                                    )
                        else:
                            nc.scalar.activation(ew[:, sl, :], uh[:], Exp, bias=zbias[:])
                            if relu_on_act:
                                nc.scalar.activation(rw[:, sl, :], uh[:], Relu, bias=zbias[:])
                            else:
                                nc.vector.tensor_scalar_max(rw[:, sl, :], uh[:], 0.0)
                    if MIN_ON_GPSIMD:
                        nc.gpsimd.tensor_scalar_min(tw[:], ew[:], 1.0)
                    else:
                        nc.vector.tensor_scalar_min(tw[:], ew[:], 1.0)
                    rts[g] = (rw, tw)

                    # ---- dense2 deferred by one group for pipelining
                    if g >= 3 and g % 2 == 1:
                        dense2_pair((g - 3) // 2)
                dense2_pair(3)

                # ---- transpose [128q, 512b] -> 4x [128b, 128q], store
                ot = ps_t.tile([128, 4, 128], F32, tag="tp")
                for k in range(4):
                    nc.tensor.transpose(ot[:, k, :], outq[:, 128 * k:128 * (k + 1)], identf[:])
                ob = outb_pool.tile([128, 4, 128], F32, tag="outb")
                nc.vector.tensor_copy(ob[:], ot[:])
                nc.sync.dma_start(
                    out_d[NB * n:NB * (n + 1), :].rearrange("(k p) q -> p k q", p=128),
                    ob[:],
                )

    return nc


_CACHE = {}


def _get_nc(bc, has_b1):
    key = (bc, has_b1)
    if key not in _CACHE:
        _CACHE[key] = _build(bc, has_b1)
    return _CACHE[key]


def kernel(x, W1, b1, gamma, beta, mov_mean, mov_var, W2, b2):
    x = np.asarray(x, np.float32).reshape(-1, C)
    B = x.shape[0]
    w1bd, w2t, bfin, b1sb, has_b1 = _host_pack(
        W1, b1, gamma, beta, mov_mean, mov_var, W2, b2
    )

    bc = B // NCORES
    nc = _get_nc(bc, has_b1)

    in_maps = [
        {
            "x": np.ascontiguousarray(x[i * bc:(i + 1) * bc]),
            "w1bd": w1bd,
            "w2t": w2t,
            "bfin": bfin,
            "b1sb": b1sb,
            "ident": IDENT,
        }
        for i in range(NCORES)
    ]
    res = run_bass_kernel_spmd(nc, in_maps, list(range(NCORES)))
    kernel._last_results = res
    out = np.concatenate([res.results[i]["out"] for i in range(NCORES)], axis=0)
    return out.astype(np.float32)
